# revision 1
# baseline (speedup 1.0000x reference)
"""Trainium2 Bass kernel for nn_DE_NN_35820027249305 (dense_mlp, memory regime).

Reference computation (per particle l, per batch element b, x = X[l,0,b]):
    y = w4 @ relu(W3 @ relu(W2 @ relu(w1 * x)))
The MLP has no biases, so each particle's scalar->scalar map is positively
homogeneous: f(x) = x*f(1) for x>=0 and f(x) = -x*f(-1) for x<0.  The weights
therefore fold (on host, 44*72 flops) into two per-particle slopes a = f(1),
b = -f(-1), and the kernel becomes the purely memory-bound elementwise stream
    y = a*x + (b-a)*min(x, 0)

Device kernel per core (batch-sharded, 400000/8 = 50000 per core, all 44
particles): data laid out as (880, 2500) so every SBUF partition row maps to
exactly one particle; per tile [128, 2500], both ops on the vector engine
with per-partition scalar coefficients:
    u = min(x, 0) * c2[p]        (TensorScalar, min+mult fused)
    y = (x * c1[p]) + u          (scalar_tensor_tensor, mult+add fused)
DMA in/out 17.6 MB per core => ~49 us HBM roofline at ~358 GB/s/core.
Measured ~58 us NEFF exec: ~6 us fixed engine-init preamble + ~2.5 us first
DMA descriptor latency + ~48 us DMA at line rate + ~2 us tail barrier.
"""

import time
from contextlib import ExitStack

import numpy as np

import concourse.bass as bass
import concourse.mybir as mybir
from concourse.bass_utils import run_bass_kernel_spmd

# Problem constants (hardcoded per the harness contract).
N_PART = 44          # particles
BATCH = 400000       # full batch
N_CORES = 8
B_CORE = BATCH // N_CORES      # 50000 batch elements per core
F = 2500                       # free-dim tile width
RPP = B_CORE // F              # rows per particle = 20
ROWS = N_PART * RPP            # 880 rows per core
P = 128
NT = (ROWS + P - 1) // P       # 7 tiles (last has 112 rows)
NBUF = 4                       # buffer slots per stream (x / u / y)

_CACHED = {}


def _build_kernel():
    """Raw-bass kernel with explicit semaphores.

    The walrus build in this container allows at most ONE semaphore wait
    embedded per instruction, so Tile's auto-generated multi-wait sync does
    not compile.  Raw bass lets us issue standalone wait_ge instructions
    (EventSemaphore ops, one wait each) and keep every DMA/compute
    instruction wait-free.

    Engine programs:
      SP  (nc.sync):   coefficient DMA + x-tile loads      (qSPDynamicHW)
      ACT (nc.scalar): y-tile stores                        (qActDynamicHW)
      DVE (nc.vector): per tile
            u = min(x, 0) * c2          -- TensorScalar (min, mult)
            y = (x * c1) + u            -- scalar_tensor_tensor (mult, add)
    """
    if "nc" in _CACHED:
        return _CACHED["nc"]
    f32 = mybir.dt.float32
    nc = bass.Bass()
    # Strip the init-time all-engine barrier (per-engine Drain +
    # EventSemaphore) that Bass.__init__ emits after the const memsets.  This
    # kernel never reads the const tensors and does all cross-engine ordering
    # through its own semaphores, so the barrier only adds ~3-6 us of
    # engine-start skew before the first DMA trigger.
    main = nc.m.functions[0].blocks[0]
    main.instructions = [
        i
        for i in main.instructions
        if type(i).__name__ not in ("InstDrain", "InstEventSemaphore")
    ]
    x_in = nc.declare_dram_parameter("x_in", [ROWS, F], f32, isOutput=False)
    cm = nc.declare_dram_parameter("cm", [P, 2 * NT], f32, isOutput=False)
    y_out = nc.declare_dram_parameter("y_out", [ROWS, F], f32, isOutput=True)

    ctx = ExitStack()
    with ctx:
        cms = ctx.enter_context(nc.sbuf_tensor("cms", [P, 2 * NT], f32))
        xb = [
            ctx.enter_context(nc.sbuf_tensor(f"xb{i}", [P, F], f32))
            for i in range(NBUF)
        ]
        ub = [
            ctx.enter_context(nc.sbuf_tensor(f"ub{i}", [P, F], f32))
            for i in range(NBUF)
        ]
        yb = [
            ctx.enter_context(nc.sbuf_tensor(f"yb{i}", [P, F], f32))
            for i in range(NBUF)
        ]
        s_cm = ctx.enter_context(nc.semaphore("s_cm"))
        s_load = ctx.enter_context(nc.semaphore("s_load"))
        s_comp = ctx.enter_context(nc.semaphore("s_comp"))
        s_store = ctx.enter_context(nc.semaphore("s_store"))

        # All three engine streams live in the main block — no nc.Block(), so
        # no branch into body blocks (and no ~1us IRAM fetch at the branch).
        # Per-engine program order is the emission order below.

        # SP stream: x-tile loads.  First tile's load is split across both
        # HWDGE rings (SP here, ACT below) so the two descriptor generators
        # work in parallel.
        sync = nc.sync
        sync.dma_start(xb[0][: P // 2], x_in[0 : P // 2, :]).then_inc(s_load, 16)
        for t in range(1, NT):
            if t >= NBUF:
                # xb/ub slot free once compute of tile t-NBUF finished
                sync.wait_ge(s_comp, t - NBUF + 1)
            r0 = t * P
            p = min(P, ROWS - r0)
            sync.dma_start(xb[t % NBUF][:p], x_in[r0 : r0 + p, :]).then_inc(
                s_load, 16
            )

        # ACT stream: coefficient load, second half of x tile 0, y stores.
        scalar = nc.scalar
        scalar.dma_start(cms[:], cm[:]).then_inc(s_cm, 16)
        scalar.dma_start(xb[0][P // 2 :], x_in[P // 2 : P, :]).then_inc(s_cm, 16)
        for t in range(NT):
            r0 = t * P
            p = min(P, ROWS - r0)
            scalar.wait_ge(s_comp, t + 1)  # y tile t ready
            scalar.dma_start(y_out[r0 : r0 + p, :], yb[t % NBUF][:p]).then_inc(
                s_store, 16
            )
        scalar.wait_ge(s_store, 16 * NT)  # all outputs landed in HBM

        # DVE stream: the two fused elementwise ops per tile.
        vector = nc.vector
        # cm load + second half of x tile 0 (both on the ACT ring, FIFO)
        vector.wait_ge(s_cm, 32)
        for t in range(NT):
            i = t % NBUF
            p = min(P, ROWS - t * P)
            vector.wait_ge(s_load, 16 * (t + 1))  # x tile t in SBUF
            if t >= NBUF:
                # yb slot drained by store of tile t-NBUF
                vector.wait_ge(s_store, 16 * (t - NBUF + 1))
            # u = min(x, 0) * c2   with c2 = b - a
            vector.tensor_scalar(
                ub[i][:p],
                xb[i][:p],
                0.0,
                cms[:p, NT + t : NT + t + 1],
                mybir.AluOpType.min,
                mybir.AluOpType.mult,
            )
            vector.scalar_tensor_tensor(
                yb[i][:p],
                xb[i][:p],
                cms[:p, t : t + 1],
                ub[i][:p],
                mybir.AluOpType.mult,
                mybir.AluOpType.add,
            ).then_inc(s_comp, 1)

        # End-of-kernel all-engine barrier (what nc.Block() would emit).
        nc.all_engine_barrier()

    _CACHED["nc"] = nc
    return nc


def _fold_weights(lin1s, lin2s, lin3s, lin4s):
    """Collapse each particle's bias-free ReLU MLP into slopes (a, b):
    f(x) = a*x for x>0, b*x for x<0.  Returns c1 = a, c2 = b - a."""

    def f(xval):
        x = np.full((N_PART, 1, 1), xval, dtype=np.float32)
        h = np.maximum(np.einsum("lik,lkj->lij", lin1s, x), 0.0).astype(np.float32)
        h = np.maximum(np.einsum("lik,lkj->lij", lin2s, h), 0.0).astype(np.float32)
        h = np.maximum(np.einsum("lik,lkj->lij", lin3s, h), 0.0).astype(np.float32)
        return np.einsum("lik,lkj->lij", lin4s, h)[:, 0, 0].astype(np.float32)

    a = f(1.0)
    b = -f(-1.0)
    # y = c1*x + c2*min(x, 0)  with c1 = a, c2 = b - a
    c1 = a.astype(np.float32)
    c2 = (b - a).astype(np.float32)
    return c1, c2


def _make_in_maps(X, lin1s, lin2s, lin3s, lin4s):
    X = np.asarray(X, dtype=np.float32)
    c1, c2 = _fold_weights(
        np.asarray(lin1s, dtype=np.float32),
        np.asarray(lin2s, dtype=np.float32),
        np.asarray(lin3s, dtype=np.float32),
        np.asarray(lin4s, dtype=np.float32),
    )

    # Per-partition-row coefficient maps: row r of the (ROWS, F) layout holds
    # data of particle r // RPP.  Same for every core (batch sharding).
    row_particle = np.arange(NT * P) // RPP          # len 896; rows >= 880 pad
    row_particle = np.minimum(row_particle, N_PART - 1)
    c1_map = c1[row_particle].reshape(NT, P).T  # [P, NT]
    c2_map = c2[row_particle].reshape(NT, P).T
    cm_map = np.ascontiguousarray(
        np.concatenate([c1_map, c2_map], axis=1), dtype=np.float32
    )  # [P, 2*NT]

    in_maps = []
    for c in range(N_CORES):
        shard = np.ascontiguousarray(
            X[:, 0, c * B_CORE : (c + 1) * B_CORE]
        ).reshape(ROWS, F)
        in_maps.append({"x_in": shard, "cm": cm_map})
    return in_maps


def _gather(results):
    out = np.empty((N_PART, 1, BATCH), dtype=np.float32)
    for c in range(N_CORES):
        y = results[c]["y_out"].reshape(N_PART, B_CORE)
        out[:, 0, c * B_CORE : (c + 1) * B_CORE] = y
    return out


def kernel(X, lin1s, lin2s, lin3s, lin4s):
    nc = _build_kernel()
    in_maps = _make_in_maps(X, lin1s, lin2s, lin3s, lin4s)
    try:
        res = run_bass_kernel_spmd(nc, in_maps, core_ids=list(range(N_CORES)))
    except Exception:
        # Transient NRT_EXEC_UNIT_UNRECOVERABLE wedges have been observed to
        # clear after a few minutes; give the device one chance to recover.
        time.sleep(150)
        res = run_bass_kernel_spmd(nc, in_maps, core_ids=list(range(N_CORES)))
    return _gather(res.results)



# revision 2
# speedup vs baseline: 1.4573x; 1.4573x over previous
"""Trainium2 Bass kernel for nn_DE_NN_35820027249305 (dense_mlp, memory regime).

Reference computation (per particle l, per batch element b, x = X[l,0,b]):
    y = w4 @ relu(W3 @ relu(W2 @ relu(w1 * x)))
The MLP has no biases, so each particle's scalar->scalar map is positively
homogeneous: f(x) = x*f(1) for x>=0 and f(x) = -x*f(-1) for x<0.  The weights
fold (on host, 44*72 flops) into two per-particle slopes a = f(1), b = -f(-1)
and the kernel is the purely memory-bound elementwise stream
    y = a*max(x, 0) + b*min(x, 0)

The stream is quantized to bf16 on both sides (norm rel-err ~2e-3, gate is
2e-2), halving HBM traffic vs f32: 4.4 MB in + 4.4 MB out per core.

Device kernel per core (batch-sharded, 400000/8 = 50000 per core, all 44
particles): data laid out as (880, 2500) bf16 so every SBUF partition row
maps to exactly one particle; 7 row-tiles of [128, 2500].  Per tile, three
DVE ops that all run in the fast 2-byte perf modes (4x/4x/2x):
    u = min(x, 0) * b[p]         (TensorScalar, 4x)
    v = max(x, 0) * a[p]         (TensorScalar, 4x)
    y = u + v                    (TensorTensor, 2x)
Loads and stores are interleaved across the two HWDGE rings (SP and ACT) so
both descriptor queues stay busy (a single queue caps at ~244 GB/s; two
sustain ~380 GB/s aggregate).  All 7 input tiles have dedicated SBUF buffers
so every load is queued up front with no recycling waits.  The last tile's
compute+store is split into two column halves to shorten the drain tail.
"""

import time
from contextlib import ExitStack

import ml_dtypes
import numpy as np

import concourse.bass as bass
import concourse.mybir as mybir
from concourse.bass_utils import run_bass_kernel_spmd

# Problem constants (hardcoded per the harness contract).
N_PART = 44          # particles
BATCH = 400000       # full batch
N_CORES = 8
B_CORE = BATCH // N_CORES      # 50000 batch elements per core
F = 2500                       # free-dim tile width
RPP = B_CORE // F              # rows per particle = 20
ROWS = N_PART * RPP            # 880 rows per core
P = 128
NT = (ROWS + P - 1) // P       # 7 tiles (last has 112 rows)

BF16 = ml_dtypes.bfloat16

_CACHED = {}


def _build_kernel():
    """Raw-bass kernel with explicit semaphores.

    The walrus build in this container allows at most ONE semaphore wait
    embedded per instruction, so standalone wait_ge instructions are used and
    every DMA/compute instruction is wait-free.

    Engine programs:
      SP  (nc.sync):   cm load, x0/x2/x4/x6 loads, y1/y3/y5 stores
      ACT (nc.scalar): x1/x3/x5 loads, y0/y2/y4/y6a/y6b stores
      DVE (nc.vector): per tile, three fast-mode elementwise ops
    """
    if "nc" in _CACHED:
        return _CACHED["nc"]
    f32 = mybir.dt.float32
    bf16 = mybir.dt.bfloat16
    nc = bass.Bass()
    # Strip the init-time all-engine barrier (per-engine Drain +
    # EventSemaphore) that Bass.__init__ emits after the const memsets.  This
    # kernel never reads the const tensors and does all cross-engine ordering
    # through its own semaphores, so the barrier only adds engine-start skew
    # before the first DMA trigger.
    main = nc.m.functions[0].blocks[0]
    main.instructions = [
        i
        for i in main.instructions
        if type(i).__name__ not in ("InstDrain", "InstEventSemaphore")
    ]
    x_in = nc.declare_dram_parameter("x_in", [ROWS, F], bf16, isOutput=False)
    cm = nc.declare_dram_parameter("cm", [P, 2 * NT], f32, isOutput=False)
    y_out = nc.declare_dram_parameter("y_out", [ROWS, F], bf16, isOutput=True)

    sp_tiles = [0, 2, 4, 6]
    act_tiles = [1, 3, 5]

    ctx = ExitStack()
    with ctx:
        cms = ctx.enter_context(nc.sbuf_tensor("cms", [P, 2 * NT], f32))
        xb = [
            ctx.enter_context(nc.sbuf_tensor(f"xb{i}", [P, F], bf16))
            for i in range(NT)
        ]
        yb = [
            ctx.enter_context(nc.sbuf_tensor(f"yb{i}", [P, F], bf16))
            for i in range(NT)
        ]
        ub = ctx.enter_context(nc.sbuf_tensor("ub", [P, F], bf16))
        vb = ctx.enter_context(nc.sbuf_tensor("vb", [P, F], bf16))
        s_cm = ctx.enter_context(nc.semaphore("s_cm"))
        s_la = ctx.enter_context(nc.semaphore("s_la"))
        s_lb = ctx.enter_context(nc.semaphore("s_lb"))
        s_comp = ctx.enter_context(nc.semaphore("s_comp"))
        s_store = ctx.enter_context(nc.semaphore("s_store"))

        def rows(t):
            return min(P, ROWS - t * P)

        # SP stream: coefficients first (tiny), then its share of x tiles all
        # queued up front, then its share of y stores.
        sync = nc.sync
        sync.dma_start(cms[:], cm[:]).then_inc(s_cm, 16)
        for t in sp_tiles:
            r0, p = t * P, rows(t)
            sync.dma_start(xb[t][:p], x_in[r0 : r0 + p, :]).then_inc(s_la, 16)
        for t in act_tiles:  # y1, y3, y5
            r0, p = t * P, rows(t)
            sync.wait_ge(s_comp, t + 1)
            sync.dma_start(y_out[r0 : r0 + p, :], yb[t][:p]).then_inc(
                s_store, 16
            )

        # ACT stream: its share of x tiles, then the remaining y stores (the
        # last tile as two column-half chunks).
        scalar = nc.scalar
        for t in act_tiles:
            r0, p = t * P, rows(t)
            scalar.dma_start(xb[t][:p], x_in[r0 : r0 + p, :]).then_inc(
                s_lb, 16
            )
        for t in [0, 2, 4]:
            r0, p = t * P, rows(t)
            scalar.wait_ge(s_comp, t + 1)
            scalar.dma_start(y_out[r0 : r0 + p, :], yb[t][:p]).then_inc(
                s_store, 16
            )
        r0, p = 6 * P, rows(6)
        half = F // 2
        scalar.wait_ge(s_comp, 7)
        scalar.dma_start(
            y_out[r0 : r0 + p, :half], yb[6][:p, :half]
        ).then_inc(s_store, 16)
        scalar.wait_ge(s_comp, 8)
        scalar.dma_start(
            y_out[r0 : r0 + p, half:], yb[6][:p, half:]
        ).then_inc(s_store, 16)
        scalar.wait_ge(s_store, 16 * (NT + 1))  # all 8 stores landed in HBM

        # DVE stream: three fast-mode ops per tile.
        vector = nc.vector
        vector.wait_ge(s_cm, 16)

        def compute(t, p, c0, c1):
            i = t
            vector.tensor_scalar(
                ub[:p, c0:c1],
                xb[i][:p, c0:c1],
                0.0,
                cms[:p, NT + t : NT + t + 1],
                mybir.AluOpType.min,
                mybir.AluOpType.mult,
            )
            vector.tensor_scalar(
                vb[:p, c0:c1],
                xb[i][:p, c0:c1],
                0.0,
                cms[:p, t : t + 1],
                mybir.AluOpType.max,
                mybir.AluOpType.mult,
            )
            vector.tensor_tensor(
                yb[i][:p, c0:c1],
                ub[:p, c0:c1],
                vb[:p, c0:c1],
                mybir.AluOpType.add,
            ).then_inc(s_comp, 1)

        na = nb = 0
        for t in range(NT):
            p = rows(t)
            if t in sp_tiles:
                na += 1
                vector.wait_ge(s_la, 16 * na)
            else:
                nb += 1
                vector.wait_ge(s_lb, 16 * nb)
            if t < NT - 1:
                compute(t, p, 0, F)
            else:
                compute(t, p, 0, F // 2)
                compute(t, p, F // 2, F)

        # End-of-kernel all-engine barrier (what nc.Block() would emit).
        nc.all_engine_barrier()

    _CACHED["nc"] = nc
    return nc


def _fold_weights(lin1s, lin2s, lin3s, lin4s):
    """Collapse each particle's bias-free ReLU MLP into slopes (a, b):
    f(x) = a*x for x>0, b*x for x<0."""

    def f(xval):
        x = np.full((N_PART, 1, 1), xval, dtype=np.float32)
        h = np.maximum(np.einsum("lik,lkj->lij", lin1s, x), 0.0).astype(np.float32)
        h = np.maximum(np.einsum("lik,lkj->lij", lin2s, h), 0.0).astype(np.float32)
        h = np.maximum(np.einsum("lik,lkj->lij", lin3s, h), 0.0).astype(np.float32)
        return np.einsum("lik,lkj->lij", lin4s, h)[:, 0, 0].astype(np.float32)

    a = f(1.0)
    b = -f(-1.0)
    return a.astype(np.float32), b.astype(np.float32)


def _make_in_maps(X, lin1s, lin2s, lin3s, lin4s):
    X = np.asarray(X, dtype=np.float32)
    a, b = _fold_weights(
        np.asarray(lin1s, dtype=np.float32),
        np.asarray(lin2s, dtype=np.float32),
        np.asarray(lin3s, dtype=np.float32),
        np.asarray(lin4s, dtype=np.float32),
    )

    # Per-partition-row coefficient maps: row r of the (ROWS, F) layout holds
    # data of particle r // RPP.  Same for every core (batch sharding).
    row_particle = np.arange(NT * P) // RPP          # len 896; rows >= 880 pad
    row_particle = np.minimum(row_particle, N_PART - 1)
    a_map = a[row_particle].reshape(NT, P).T  # [P, NT]
    b_map = b[row_particle].reshape(NT, P).T
    cm_map = np.ascontiguousarray(
        np.concatenate([a_map, b_map], axis=1), dtype=np.float32
    )  # [P, 2*NT]

    in_maps = []
    for c in range(N_CORES):
        shard = (
            np.ascontiguousarray(X[:, 0, c * B_CORE : (c + 1) * B_CORE])
            .reshape(ROWS, F)
            .astype(BF16)
        )
        in_maps.append({"x_in": shard, "cm": cm_map})
    return in_maps


def _gather(results):
    out = np.empty((N_PART, 1, BATCH), dtype=np.float32)
    for c in range(N_CORES):
        y = results[c]["y_out"].astype(np.float32).reshape(N_PART, B_CORE)
        out[:, 0, c * B_CORE : (c + 1) * B_CORE] = y
    return out


def kernel(X, lin1s, lin2s, lin3s, lin4s):
    nc = _build_kernel()
    in_maps = _make_in_maps(X, lin1s, lin2s, lin3s, lin4s)
    try:
        res = run_bass_kernel_spmd(nc, in_maps, core_ids=list(range(N_CORES)))
    except Exception:
        # Transient NRT_EXEC_UNIT_UNRECOVERABLE wedges have been observed to
        # clear after a few minutes; give the device one chance to recover.
        time.sleep(150)
        res = run_bass_kernel_spmd(nc, in_maps, core_ids=list(range(N_CORES)))
    return _gather(res.results)


# revision 4
# speedup vs baseline: 1.5115x; 1.0371x over previous
"""Trainium2 Bass kernel for nn_DE_NN_35820027249305 (dense_mlp, memory regime).

Reference computation (per particle l, per batch element b, x = X[l,0,b]):
    y = w4 @ relu(W3 @ relu(W2 @ relu(w1 * x)))
The MLP has no biases, so each particle's scalar->scalar map is positively
homogeneous: f(x) = x*f(1) for x>=0 and f(x) = -x*f(-1) for x<0.  The weights
fold (on host, 44*72 flops) into two per-particle slopes a = f(1), b = -f(-1)
and the kernel is the purely memory-bound elementwise stream
    y = a*max(x, 0) + b*min(x, 0)

The stream is quantized to bf16 on both sides (norm rel-err ~2e-3, gate is
2e-2), halving HBM traffic vs f32: 4.4 MB in + 4.4 MB out per core.  Data is
laid out as (880, 2500) bf16 per core so every SBUF partition row maps to one
particle; 7 row-tiles of [128, 2500].

Compute is split across two engines so neither paces the DMA stream:
  - DVE tiles: three ops, all in fast 2-byte DVE perf modes (4x/4x/2x):
        u = min(x,0)*b[p];  v = max(x,0)*a[p];  y = u + v
  - ACT (hybrid) tiles: y = a[p] * prelu(x, alpha[p]) with alpha = b/a
    (sign-safe for any a since prelu branches on the sign of x itself);
    the prelu runs on the scalar engine, the per-partition multiply is a
    single 4x-mode TensorScalar on DVE.  Exactly-zero slopes a are clamped
    to signed 1e-8 (alpha = b/1e-8), which reproduces b*x on the negative
    branch exactly and ~0 on the positive branch.

Loads and stores are interleaved across the two HWDGE rings (SP and ACT) so
both descriptor queues stay busy (a single queue caps at ~244 GB/s; two
sustain ~420 GB/s aggregate).  All input tiles have dedicated SBUF buffers
and are queued up front; tile 0's load is split into three column chunks so
the first DVE op starts ~4 us earlier; the last tile's multiply and store
are split into column halves to shorten the drain tail.
"""

import time
from contextlib import ExitStack

import ml_dtypes
import numpy as np

import concourse.bass as bass
import concourse.mybir as mybir
from concourse.bass_utils import run_bass_kernel_spmd

# Problem constants (hardcoded per the harness contract).
N_PART = 44          # particles
BATCH = 400000       # full batch
N_CORES = 8
B_CORE = BATCH // N_CORES      # 50000 batch elements per core
F = 2500                       # free-dim tile width
RPP = B_CORE // F              # rows per particle = 20
ROWS = N_PART * RPP            # 880 rows per core
P = 128
NT = (ROWS + P - 1) // P       # 7 tiles (last has 112 rows)

HYBRID = (1, 3, 5, 6)          # tiles computed via ACT prelu + DVE multiply
X0_SPLITS = (625, 1250, 2500)  # col boundaries of tile 0's chunked load
T6_SPLIT = 1250                # col split of the last tile's multiply/store

BF16 = ml_dtypes.bfloat16

_CACHED = {}


def _build_kernel():
    """Raw-bass kernel with explicit semaphores (one wait per instruction,
    standalone wait_ge ops; every DMA/compute instruction is wait-free).

    Engine programs:
      SP  (nc.sync):   x0a/x0b/x0c, x2, x4, x6 loads; y1, y3, y6a stores
      ACT (nc.scalar): cm load; x1, x3, x5 loads; prelu for hybrid tiles;
                       y0, y2, y4, y5, y6b stores
      DVE (nc.vector): 3-op tiles 0/2/4, per-partition multiply for hybrids
    """
    if "nc" in _CACHED:
        return _CACHED["nc"]
    f32 = mybir.dt.float32
    bf16 = mybir.dt.bfloat16
    nc = bass.Bass()
    # Strip the init-time all-engine barrier (per-engine Drain +
    # EventSemaphore) that Bass.__init__ emits after the const memsets; all
    # cross-engine ordering here goes through explicit semaphores.
    main = nc.m.functions[0].blocks[0]
    main.instructions = [
        i
        for i in main.instructions
        if type(i).__name__ not in ("InstDrain", "InstEventSemaphore")
    ]
    x_in = nc.declare_dram_parameter("x_in", [ROWS, F], bf16, isOutput=False)
    cm = nc.declare_dram_parameter("cm", [P, 3 * NT], f32, isOutput=False)
    y_out = nc.declare_dram_parameter("y_out", [ROWS, F], bf16, isOutput=True)

    ctx = ExitStack()
    with ctx:
        cms = ctx.enter_context(nc.sbuf_tensor("cms", [P, 3 * NT], f32))
        xb = [
            ctx.enter_context(nc.sbuf_tensor(f"xb{i}", [P, F], bf16))
            for i in range(NT)
        ]
        yb = [
            ctx.enter_context(nc.sbuf_tensor(f"yb{i}", [P, F], bf16))
            for i in range(NT)
        ]
        wb = {
            t: ctx.enter_context(nc.sbuf_tensor(f"wb{t}", [P, F], bf16))
            for t in HYBRID
        }
        ub = ctx.enter_context(nc.sbuf_tensor("ub", [P, F], bf16))
        vb = ctx.enter_context(nc.sbuf_tensor("vb", [P, F], bf16))
        s_cm = ctx.enter_context(nc.semaphore("s_cm"))
        s_la = ctx.enter_context(nc.semaphore("s_la"))
        s_lb = ctx.enter_context(nc.semaphore("s_lb"))
        s_act = ctx.enter_context(nc.semaphore("s_act"))
        s_comp = ctx.enter_context(nc.semaphore("s_comp"))
        s_store = ctx.enter_context(nc.semaphore("s_store"))

        def rows(t):
            return min(P, ROWS - t * P)

        a_col = lambda t: cms[: rows(t), t : t + 1]
        b_col = lambda t: cms[: rows(t), NT + t : NT + t + 1]
        al_col = lambda t: cms[: rows(t), 2 * NT + t : 2 * NT + t + 1]

        # DVE output chunks in program order; stores wait on s_comp counts.
        #   tile0: 3 col chunks;  tiles 1..5: whole;  tile6: 2 col chunks
        comp_count = {0: 3, 1: 4, 2: 5, 3: 6, 4: 7, 5: 8, 6: 10}

        # ---- SP stream ----
        sync = nc.sync
        c_prev = 0
        for c in X0_SPLITS:
            sync.dma_start(xb[0][:, c_prev:c], x_in[0:P, c_prev:c]).then_inc(
                s_la, 16
            )
            c_prev = c
        for t in (2, 4, 6):
            r0, p = t * P, rows(t)
            sync.dma_start(xb[t][:p], x_in[r0 : r0 + p, :]).then_inc(s_la, 16)
        for t in (1, 3):
            r0, p = t * P, rows(t)
            sync.wait_ge(s_comp, comp_count[t])
            sync.dma_start(y_out[r0 : r0 + p, :], yb[t][:p]).then_inc(
                s_store, 16
            )
        r0, p = 6 * P, rows(6)
        sync.wait_ge(s_comp, 9)  # tile6 first chunk
        sync.dma_start(
            y_out[r0 : r0 + p, :T6_SPLIT], yb[6][:p, :T6_SPLIT]
        ).then_inc(s_store, 16)

        # ---- ACT stream ----
        scalar = nc.scalar
        scalar.dma_start(cms[:], cm[:]).then_inc(s_cm, 16)
        for t in (1, 3, 5):
            r0, p = t * P, rows(t)
            scalar.dma_start(xb[t][:p], x_in[r0 : r0 + p, :]).then_inc(
                s_lb, 16
            )

        def prelu(t, lb_count, la_count):
            p = rows(t)
            if la_count:
                scalar.wait_ge(s_la, 16 * la_count)
            else:
                scalar.wait_ge(s_lb, 16 * lb_count)
            scalar.activation(
                wb[t][:p],
                xb[t][:p],
                mybir.ActivationFunctionType.Prelu,
                alpha=al_col(t),
            ).then_inc(s_act, 1)

        def store(t, c0=0, c1=F, cnt=None):
            r0, p = t * P, rows(t)
            scalar.wait_ge(s_comp, cnt if cnt is not None else comp_count[t])
            scalar.dma_start(
                y_out[r0 : r0 + p, c0:c1], yb[t][:p, c0:c1]
            ).then_inc(s_store, 16)

        prelu(1, lb_count=1, la_count=0)   # x1
        store(0)
        prelu(3, lb_count=2, la_count=0)   # x3
        store(2)
        prelu(5, lb_count=3, la_count=0)   # x5
        prelu(6, lb_count=0, la_count=6)   # x6 (6th DMA on s_la)
        store(4)
        store(5)
        store(6, c0=T6_SPLIT, c1=F, cnt=10)
        scalar.wait_ge(s_store, 16 * 8)  # all 8 stores landed in HBM

        # ---- DVE stream ----
        vector = nc.vector
        vector.wait_ge(s_cm, 16)

        def dve_full(t, c0, c1):
            p = rows(t)
            vector.tensor_scalar(
                ub[:p, c0:c1],
                xb[t][:p, c0:c1],
                0.0,
                b_col(t),
                mybir.AluOpType.min,
                mybir.AluOpType.mult,
            )
            vector.tensor_scalar(
                vb[:p, c0:c1],
                xb[t][:p, c0:c1],
                0.0,
                a_col(t),
                mybir.AluOpType.max,
                mybir.AluOpType.mult,
            )
            vector.tensor_tensor(
                yb[t][:p, c0:c1],
                ub[:p, c0:c1],
                vb[:p, c0:c1],
                mybir.AluOpType.add,
            ).then_inc(s_comp, 1)

        def dve_mult(t, c0, c1):
            p = rows(t)
            vector.tensor_scalar_mul(
                yb[t][:p, c0:c1],
                wb[t][:p, c0:c1],
                a_col(t),
            ).then_inc(s_comp, 1)

        # tile 0: three col chunks as the loads land
        c_prev, n_la = 0, 0
        for c in X0_SPLITS:
            n_la += 1
            vector.wait_ge(s_la, 16 * n_la)
            dve_full(0, c_prev, c)
            c_prev = c
        # tile 1 (hybrid)
        vector.wait_ge(s_act, 1)
        dve_mult(1, 0, F)
        # tile 2
        vector.wait_ge(s_la, 16 * 4)
        dve_full(2, 0, F)
        # tile 3 (hybrid)
        vector.wait_ge(s_act, 2)
        dve_mult(3, 0, F)
        # tile 4
        vector.wait_ge(s_la, 16 * 5)
        dve_full(4, 0, F)
        # tile 5 (hybrid)
        vector.wait_ge(s_act, 3)
        dve_mult(5, 0, F)
        # tile 6 (hybrid, two col chunks)
        vector.wait_ge(s_act, 4)
        dve_mult(6, 0, T6_SPLIT)
        dve_mult(6, T6_SPLIT, F)

        # End-of-kernel all-engine barrier (what nc.Block() would emit).
        nc.all_engine_barrier()

    _CACHED["nc"] = nc
    return nc


def _fold_weights(lin1s, lin2s, lin3s, lin4s):
    """Collapse each particle's bias-free ReLU MLP into slopes (a, b):
    f(x) = a*x for x>0, b*x for x<0."""

    def f(xval):
        x = np.full((N_PART, 1, 1), xval, dtype=np.float32)
        h = np.maximum(np.einsum("lik,lkj->lij", lin1s, x), 0.0).astype(np.float32)
        h = np.maximum(np.einsum("lik,lkj->lij", lin2s, h), 0.0).astype(np.float32)
        h = np.maximum(np.einsum("lik,lkj->lij", lin3s, h), 0.0).astype(np.float32)
        return np.einsum("lik,lkj->lij", lin4s, h)[:, 0, 0].astype(np.float32)

    a = f(1.0)
    b = -f(-1.0)
    return a.astype(np.float32), b.astype(np.float32)


def _make_in_maps(X, lin1s, lin2s, lin3s, lin4s):
    X = np.asarray(X, dtype=np.float32)
    a, b = _fold_weights(
        np.asarray(lin1s, dtype=np.float32),
        np.asarray(lin2s, dtype=np.float32),
        np.asarray(lin3s, dtype=np.float32),
        np.asarray(lin4s, dtype=np.float32),
    )
    # prelu path: y = a_safe * prelu(x, b / a_safe); exact zeros of a are
    # clamped to signed 1e-8 (positive branch then yields ~1e-8*x ~ 0).
    a_safe = np.where(np.abs(a) < 1e-8, np.where(a < 0, -1e-8, 1e-8), a)
    alpha = (b / a_safe).astype(np.float32)

    # Per-partition-row coefficient maps: row r of the (ROWS, F) layout holds
    # data of particle r // RPP.  Same for every core (batch sharding).
    row_particle = np.arange(NT * P) // RPP          # len 896; rows >= 880 pad
    row_particle = np.minimum(row_particle, N_PART - 1)
    a_map = a_safe[row_particle].reshape(NT, P).T  # [P, NT]
    b_map = b[row_particle].reshape(NT, P).T
    al_map = alpha[row_particle].reshape(NT, P).T
    cm_map = np.ascontiguousarray(
        np.concatenate([a_map, b_map, al_map], axis=1), dtype=np.float32
    )  # [P, 3*NT]

    in_maps = []
    for c in range(N_CORES):
        shard = (
            np.ascontiguousarray(X[:, 0, c * B_CORE : (c + 1) * B_CORE])
            .reshape(ROWS, F)
            .astype(BF16)
        )
        in_maps.append({"x_in": shard, "cm": cm_map})
    return in_maps


def _gather(results):
    out = np.empty((N_PART, 1, BATCH), dtype=np.float32)
    for c in range(N_CORES):
        y = results[c]["y_out"].astype(np.float32).reshape(N_PART, B_CORE)
        out[:, 0, c * B_CORE : (c + 1) * B_CORE] = y
    return out


def kernel(X, lin1s, lin2s, lin3s, lin4s):
    nc = _build_kernel()
    in_maps = _make_in_maps(X, lin1s, lin2s, lin3s, lin4s)
    try:
        res = run_bass_kernel_spmd(nc, in_maps, core_ids=list(range(N_CORES)))
    except Exception:
        # Transient NRT_EXEC_UNIT_UNRECOVERABLE wedges have been observed to
        # clear after a few minutes; give the device one chance to recover.
        time.sleep(150)
        res = run_bass_kernel_spmd(nc, in_maps, core_ids=list(range(N_CORES)))
    return _gather(res.results)
